# revision 24
# baseline (speedup 1.0000x reference)
"""Trainium2 Bass kernel for a 2-layer BiLSTM regressor (B=256, T=512, F=32,
H=100, relu candidate/output activations, sigmoid gates) + TimeDistributed
Dense(8, relu) head.

Strategy: data-parallel over batch across 8 NeuronCores (32 sequences/core).
Sequences live on-chip transposed as [H, T*B] bf16 tensors (column = t*B+b).

The recurrence is dependency-loop-latency bound (1024 sequential cell steps).
The two directions of a layer run as two INDEPENDENT software-pipelined
chains (not lockstep-fused): each chain's per-step loop is
    PE 4 small matmuls -> ACT one sigmoid over (i,f,o) -> DVE 4 ops -> PE
and the two chains phase-shift so engine busy time hides inside the other
chain's latency segments.  Two exact algebraic simplifications shorten the
loop (c >= 0 by induction since c = sig_f*c + sig_i*relu(g) from c0=0):
    u = relu(g)*sig_i  == one DVE scalar_tensor_tensor (max 0, then mult)
    h = sig_o*relu(c)  == sig_o*c, one plain DVE mult
so no separate relu instructions exist at all.  Gate regions in PSUM are
ordered [i,f,o,g] so one ACT instruction covers all three sigmoids.

Gate pre-activations for an 8-step window land in one 2-bank PSUM tile per
chain ([H, 4*256] regions i,f,o,g); backward-direction input projections are
written in reversed time order (negative-stride matmul rhs) so both chains
use the same slot index.  Biases ride the input projections via a constant
ones row; the Dense head is a final matmul pass.  Host does the cheap
input/output transposes.
"""

import numpy as np
import ml_dtypes
from contextlib import ExitStack

H = 100          # LSTM units per direction
F = 32           # input features
NT = 8           # dense head outputs
T_FULL = 512
B_FULL = 256
N_CORES = 8
B_LOC = B_FULL // N_CORES   # 32
W = 8            # timesteps per PSUM window
# PSUM region order within the 4H axis (host permutes weight columns):
R_I, R_F, R_O, R_G = 0, 1, 2, 3

_BUILD_CACHE = {}
LAST_RESULTS = None  # BassKernelResults of the most recent run (for test.py)


def build_nc(T=T_FULL, B=B_LOC):
    """Build (and bacc-compile) the Bass program for one core."""
    key = (T, B)
    if key in _BUILD_CACHE:
        return _BUILD_CACHE[key]

    import concourse.bacc as bacc
    import concourse.tile as tile
    from concourse import mybir

    fp32 = mybir.dt.float32
    bf16 = mybir.dt.bfloat16
    MAX = mybir.AluOpType.max
    MULT = mybir.AluOpType.mult
    SIG = mybir.ActivationFunctionType.Sigmoid
    RELU = mybir.ActivationFunctionType.Relu

    assert T % W == 0
    TB = T * B
    TB1 = (T + 1) * B
    WB = W * B          # region size (256 cols)

    nc = bacc.Bacc("TRN2", target_bir_lowering=False, debug=False)

    # ---- DRAM I/O ----
    d_x = nc.dram_tensor("x_t", [F + 1, TB], bf16, kind="ExternalInput")
    d_ones = nc.dram_tensor("ones_row", [1, TB1], bf16, kind="ExternalInput")
    dw = {}
    for name, shape in [
        ("wk1f", [F + 1, 4 * H]), ("wr1f", [H, 4 * H]),
        ("wk1b", [F + 1, 4 * H]), ("wr1b", [H, 4 * H]),
        ("wk2f_f", [H + 1, 4 * H]), ("wk2f_b", [H, 4 * H]), ("wr2f", [H, 4 * H]),
        ("wk2b_f", [H + 1, 4 * H]), ("wk2b_b", [H, 4 * H]), ("wr2b", [H, 4 * H]),
        ("wdf", [H + 1, NT]), ("wdb", [H, NT]),
    ]:
        dw[name] = nc.dram_tensor(name, shape, bf16, kind="ExternalInput")
    d_y = nc.dram_tensor("y_t", [NT, TB], fp32, kind="ExternalOutput")

    with tile.TileContext(nc) as tc, ExitStack() as ctx:
        persist = ctx.enter_context(tc.tile_pool(name="persist", bufs=1))
        psum = ctx.enter_context(tc.tile_pool(name="psum", bufs=2, space="PSUM"))
        small = ctx.enter_context(tc.tile_pool(name="small", bufs=4))
        outp = ctx.enter_context(tc.tile_pool(name="outp", bufs=4))

        # ---- persistent SBUF tensors ----
        sb_x = persist.tile([F + 1, TB], bf16, name="sb_x")
        # fwd sequences keep h(phys t) at block t+1 (block 0 = zeros);
        # bwd sequences keep h(phys t) at block t (block T = zeros).
        sb_h1f = persist.tile([H + 1, TB1], bf16, name="sb_h1f")
        sb_h1b = persist.tile([H, TB1], bf16, name="sb_h1b")
        sb_h2f = persist.tile([H + 1, TB1], bf16, name="sb_h2f")
        sb_h2b = persist.tile([H, TB1], bf16, name="sb_h2b")
        sbw = {}
        for name, d in dw.items():
            sbw[name] = persist.tile(list(d.shape), bf16, name="sb_" + name)
            nc.sync.dma_start(out=sbw[name], in_=d.ap())
        nc.sync.dma_start(out=sb_x, in_=d_x.ap())
        # ones rows (bias folding) via DMA (vector memset over 16K cols is slow)
        nc.sync.dma_start(out=sb_h1f[H:H + 1, :], in_=d_ones.ap())
        nc.sync.dma_start(out=sb_h2f[H:H + 1, :], in_=d_ones.ap())

        # zero-filled "previous [h1,h2]" staging for each layer's first slot.
        # Chain B gets its own copy, rewritten per layer from chain F's
        # slot-0 sigmoid (times zero): the data dependency staggers B's
        # pipeline phase ~700ns behind F so the two chains' engine use
        # interleaves instead of colliding.
        z_stag = persist.tile([H, 2, B], bf16, name="z_stag")
        nc.vector.memset(z_stag, 0.0)
        z_stag_b = persist.tile([H, 2, B], bf16, name="z_stag_b")
        nc.vector.memset(z_stag_b, 0.0)
        cst = {}
        for lname in ("c1f", "c1b", "c2f", "c2b"):
            cst[lname] = persist.tile([H, B], bf16, name=lname)
            nc.vector.memset(cst[lname], 0.0)

        XCH = 2          # xproj chunk: 2 blocks (64 cols) per matmul
        NCH = W // XCH   # chunks per window region

        def xrhs(src, K, coff, t_lo, reverse, j):
            """Chunk j (XCH blocks) of the window rhs [K, XCH*B]; block
            order reversed for bwd.  Small chunks keep the PE queue
            preemptible so xproj never delays recurrent matmuls long."""
            sl = src[0:K, coff + t_lo * B: coff + (t_lo + W) * B]
            v = sl.rearrange("p (w b) -> p w b", b=B)
            if reverse:
                v = v[:, ::-1, :]
            return v[:, j * XCH:(j + 1) * XCH, :]

        def bilstm(xsrc_f, xsrc_b, wr_f, wr_b, out_f, out_b, c_f, c_b):
            """One bidirectional layer; fwd/bwd as two pipelined chains."""
            nwin = T // W
            chains = [
                dict(cn="f", xsrc=xsrc_f, wr=wr_f, c=c_f, rev=False,
                     stag=z_stag[:, :]),
                dict(cn="b", xsrc=xsrc_b, wr=wr_b, c=c_b, rev=True,
                     stag=z_stag_b[:, :]),
            ]

            def xproj_thunks(w):
                """Input-projection matmul closures for window w.  One PSUM
                tile per chain, regions [i,f,o,g]; per-bank start flags.
                Tiles allocated lazily at first call (emission time)."""
                t0f = w * W
                thb = T - 1 - w * W
                tiles = {}

                def get_tile(cn):
                    if cn not in tiles:
                        tiles[cn] = psum.tile([H, 4 * WB], fp32, bufs=2,
                                              tag="g" + cn, name="g" + cn)
                    return tiles[cn]

                per_chain = []
                for ch in chains:
                    cn = ch["cn"]
                    t_lo = t0f if not ch["rev"] else thb - W + 1
                    started = set()
                    lst = []
                    for r in range(4):
                        for j in range(NCH):
                            off = r * WB + j * XCH * B
                            bank = off // 512
                            for wk, src, K, coff in ch["xsrc"]:
                                first = bank not in started
                                started.add(bank)

                                def thunk(r=r, off=off, j=j, wk=wk, src=src,
                                          K=K, coff=coff, first=first,
                                          t_lo=t_lo, cn=cn, rev=ch["rev"]):
                                    reg = get_tile(cn)
                                    nc.tensor.matmul(
                                        out=reg[:, off:off + XCH * B],
                                        lhsT=wk[:, r * H:(r + 1) * H],
                                        rhs=xrhs(src, K, coff, t_lo, rev, j),
                                        start=first, stop=False,
                                        skip_group_check=True)
                                lst.append(thunk)
                    per_chain.append(lst)
                # interleave F,B so each slot's share touches both chains
                thunks = []
                for pair in zip(*per_chain):
                    thunks.extend(pair)
                return get_tile, thunks

            get_tile, thunks = xproj_thunks(0)
            for th in thunks:
                th()
            for w in range(nwin):
                t0f = w * W                    # fwd slot s -> phys t0f + s
                thb = T - 1 - w * W            # bwd slot s -> phys thb - s
                if w + 1 < nwin:
                    next_get_tile, next_thunks = xproj_thunks(w + 1)
                else:
                    next_get_tile, next_thunks = None, []
                per_slot = (len(next_thunks) + W - 1) // W
                # window-sized [h1,h2] pair staging: feeds next slot's
                # matmuls; summed once per window into wstag then DMA'd to
                # the big sequence buffers
                wpair, wstag = {}, {}
                for ch in chains:
                    wpair[ch["cn"]] = small.tile(
                        [H, 2, WB], bf16, tag="wpair" + ch["cn"], bufs=2,
                        name="wpair" + ch["cn"])
                    wstag[ch["cn"]] = small.tile(
                        [H, WB], bf16, tag="wstag" + ch["cn"], bufs=2,
                        name="wstag" + ch["cn"])
                for k in range(W):
                    ksl = slice(k * B, (k + 1) * B)
                    for ch in chains:
                        cn = ch["cn"]
                        reg = get_tile(cn)
                        # 8 recurrent matmuls accumulate Wr^T (h1 + h2) via
                        # PSUM: h is carried split as [h1,h2] = [so*ct, so*u]
                        # (h = so*c = so*ct + so*u since c >= 0) so the
                        # critical path skips the c-add entirely.  i,f first
                        # so sig_if can start while o,g matmuls still run.
                        for r in range(4):
                            for j in range(2):
                                nc.tensor.matmul(
                                    out=reg[:, r * WB + k * B:
                                            r * WB + (k + 1) * B],
                                    lhsT=ch["wr"][:, r * H:(r + 1) * H],
                                    rhs=ch["stag"][:, j, :],
                                    start=False, stop=(j == 1),
                                    skip_group_check=True)
                        reg_v = reg.rearrange("p (r n) -> p r n", r=4)
                        # one ACT covers sigmoid of i,f,o (regions 0..2)
                        sig = small.tile([H, 3, B], bf16, tag="sig" + cn,
                                         bufs=2, name="sig" + cn)
                        nc.scalar.activation(sig, reg_v[:, 0:3, ksl], SIG)
                        if w == 0 and k == 0 and cn == "f":
                            # phase-pin: rewrite chain B's zero staging from
                            # F's slot-0 sigmoid (times zero) so B's whole
                            # pipeline starts ~700ns after F
                            nc.vector.scalar_tensor_tensor(
                                out=z_stag_b, in0=sig[:, 0:2, :], scalar=0.0,
                                in1=z_stag, op0=MULT, op1=MULT)
                        # cu = [ct, u]:  ct = sig_f * c ;  u = relu(g)*sig_i
                        cu = small.tile([H, 2, B], bf16, tag="cu" + cn,
                                        bufs=2, name="cu" + cn)
                        nc.vector.tensor_mul(cu[:, 0, :], sig[:, 1, :],
                                             ch["c"])
                        nc.vector.scalar_tensor_tensor(
                            out=cu[:, 1, :], in0=reg_v[:, R_G, ksl],
                            scalar=0.0, in1=sig[:, 0, :], op0=MAX, op1=MULT)
                        # paired h-halves: [h1,h2] = sig_o * [ct,u] (bf16)
                        # written straight into the window pair staging
                        hh = wpair[cn][:, :, ksl]
                        nc.vector.tensor_mul(
                            hh, sig[:, 2:3, :].broadcast_to([H, 2, B]), cu)
                        ch["stag"] = hh
                        # off-critical-path: c = ct + u
                        nc.vector.tensor_add(ch["c"], cu[:, 0, :],
                                             cu[:, 1, :])
                    for th in next_thunks[k * per_slot:(k + 1) * per_slot]:
                        th()
                # one window-batched dense h = h1 + h2 per chain (bf16 2x
                # DVE mode, ~24ns/slot amortized), then flush to the
                # sequence buffers: fwd slots are phys-ascending
                # (contiguous); bwd slots are phys-descending, so reverse
                # the block order on the read side
                for ch in chains:
                    cn = ch["cn"]
                    nc.vector.tensor_add(wstag[cn], wpair[cn][:, 0, :],
                                         wpair[cn][:, 1, :])
                nc.sync.dma_start(
                    out=out_f[0:H, (t0f + 1) * B:(t0f + 1 + W) * B],
                    in_=wstag["f"][:, :])
                wstag_b = wstag["b"].rearrange("p (w b) -> p w b", b=B)
                nc.sync.dma_start(
                    out=out_b[0:H, (thb - W + 1) * B:(thb + 1) * B],
                    in_=wstag_b[:, ::-1, :])
                if next_get_tile is not None:
                    get_tile = next_get_tile

        # layer 1: input = x (K = F+1 with bias row)
        bilstm(
            xsrc_f=[(sbw["wk1f"], sb_x, F + 1, 0)],
            xsrc_b=[(sbw["wk1b"], sb_x, F + 1, 0)],
            wr_f=sbw["wr1f"], wr_b=sbw["wr1b"],
            out_f=sb_h1f, out_b=sb_h1b, c_f=cst["c1f"], c_b=cst["c1b"])
        # layer 2: input = [h1f (blocks +1, ones row) ; h1b]
        bilstm(
            xsrc_f=[(sbw["wk2f_f"], sb_h1f, H + 1, B), (sbw["wk2f_b"], sb_h1b, H, 0)],
            xsrc_b=[(sbw["wk2b_f"], sb_h1f, H + 1, B), (sbw["wk2b_b"], sb_h1b, H, 0)],
            wr_f=sbw["wr2f"], wr_b=sbw["wr2b"],
            out_f=sb_h2f, out_b=sb_h2b, c_f=cst["c2f"], c_b=cst["c2b"])

        # dense head: y = relu(Wd^T [h2f;h2b] + bd) over 512-col chunks
        CH = 512
        nch = (TB + CH - 1) // CH
        for ci in range(nch):
            c0 = ci * CH
            n = min(CH, TB - c0)
            ps = psum.tile([NT, CH], fp32, bufs=2, tag="gf", name="ps_y")
            nc.tensor.matmul(out=ps[:, 0:n], lhsT=sbw["wdf"],
                             rhs=sb_h2f[0:H + 1, B + c0: B + c0 + n],
                             start=True, stop=False, skip_group_check=True)
            nc.tensor.matmul(out=ps[:, 0:n], lhsT=sbw["wdb"],
                             rhs=sb_h2b[0:H, c0: c0 + n],
                             start=False, stop=True, skip_group_check=True)
            st = outp.tile([NT, CH], fp32, tag="stage", name="st_y")
            if ci % 2 == 0:
                nc.scalar.activation(st[:, 0:n], ps[:, 0:n], RELU)
            else:
                nc.vector.tensor_single_scalar(st[:, 0:n], ps[:, 0:n], 0.0, MAX)
            nc.sync.dma_start(out=d_y.ap()[:, c0:c0 + n], in_=st[:, 0:n])

    nc.compile()
    _BUILD_CACHE[key] = nc
    return nc


# ---------------------------------------------------------------------------
# host side
# ---------------------------------------------------------------------------

def _bf16(a):
    return np.asarray(a, np.float32).astype(ml_dtypes.bfloat16)


def _perm_gates(w):
    """Reorder 4H gate columns from keras [i,f,g,o] to PSUM [i,f,o,g]."""
    w = np.asarray(w, np.float32)
    i, f, g, o = np.split(w, 4, axis=-1)
    return np.concatenate([i, f, o, g], axis=-1)


def prepare_weight_maps(Wk1f, Wr1f, b1f, Wk1b, Wr1b, b1b,
                        Wk2f, Wr2f, b2f, Wk2b, Wr2b, b2b, Wd, bd):
    def aug(w, b):
        return np.vstack([np.asarray(w, np.float32),
                          np.asarray(b, np.float32)[None, :]])
    P = _perm_gates
    m = {
        "wk1f": P(aug(Wk1f, b1f)), "wr1f": P(Wr1f),
        "wk1b": P(aug(Wk1b, b1b)), "wr1b": P(Wr1b),
        "wk2f_f": P(aug(Wk2f[:H], b2f)), "wk2f_b": P(Wk2f[H:]), "wr2f": P(Wr2f),
        "wk2b_f": P(aug(Wk2b[:H], b2b)), "wk2b_b": P(Wk2b[H:]), "wr2b": P(Wr2b),
        "wdf": aug(Wd[:H], bd), "wdb": Wd[H:],
    }
    return {k: np.ascontiguousarray(_bf16(v)) for k, v in m.items()}


def make_in_maps(x, weights, T, B):
    """x: [Btot, T, F] fp32 -> list of per-core input dicts."""
    n_cores = x.shape[0] // B
    ones = np.ones((1, (T + 1) * B), ml_dtypes.bfloat16)
    in_maps = []
    for c in range(n_cores):
        xc = np.asarray(x[c * B:(c + 1) * B], np.float32)      # [B, T, F]
        xt = xc.transpose(2, 1, 0).reshape(F, T * B)           # [F, T*B] t-major
        x_aug = np.vstack([xt, np.ones((1, T * B), np.float32)])
        in_maps.append({"x_t": np.ascontiguousarray(_bf16(x_aug)),
                        "ones_row": ones, **weights})
    return in_maps


def kernel(x, Wk1f, Wr1f, b1f, Wk1b, Wr1b, b1b,
           Wk2f, Wr2f, b2f, Wk2b, Wr2b, b2b, Wd, bd,
           trace=False):
    global LAST_RESULTS
    from concourse.bass_utils import run_bass_kernel_spmd

    Btot, T, _ = x.shape
    B = Btot // N_CORES
    nc = build_nc(T, B)
    weights = prepare_weight_maps(Wk1f, Wr1f, b1f, Wk1b, Wr1b, b1b,
                                  Wk2f, Wr2f, b2f, Wk2b, Wr2b, b2b, Wd, bd)
    in_maps = make_in_maps(x, weights, T, B)
    res = run_bass_kernel_spmd(nc, in_maps, core_ids=list(range(len(in_maps))),
                               trace=trace)
    LAST_RESULTS = res
    outs = []
    for r in res.results:
        yt = r["y_t"]                                  # [NT, T*B] fp32
        outs.append(yt.reshape(NT, T, B).transpose(2, 1, 0))   # [B, T, NT]
    return np.concatenate(outs, axis=0).astype(np.float32)


# revision 32
# speedup vs baseline: 1.1305x; 1.1305x over previous
"""Trainium2 Bass kernel for a 2-layer BiLSTM regressor (B=256, T=512, F=32,
H=100, relu candidate/output activations, sigmoid gates) + TimeDistributed
Dense(8, relu) head.

Strategy: data-parallel over batch across 8 NeuronCores (32 sequences/core).
Sequences live on-chip transposed as [H, T*B] bf16 tensors (column = t*B+b).

The recurrence is dependency-loop-latency bound (1024 sequential cell steps).
The two directions of a layer run as two INDEPENDENT software-pipelined
chains (not lockstep-fused): each chain's per-step loop is
    PE 4 small matmuls -> ACT one sigmoid over (i,f,o) -> DVE 4 ops -> PE
and the two chains phase-shift so engine busy time hides inside the other
chain's latency segments.  Two exact algebraic simplifications shorten the
loop (c >= 0 by induction since c = sig_f*c + sig_i*relu(g) from c0=0):
    u = relu(g)*sig_i  == one DVE scalar_tensor_tensor (max 0, then mult)
    h = sig_o*relu(c)  == sig_o*c, one plain DVE mult
so no separate relu instructions exist at all.  Gate regions in PSUM are
ordered [i,f,o,g] so one ACT instruction covers all three sigmoids.

Gate pre-activations for an 8-step window land in one 2-bank PSUM tile per
chain ([H, 4*256] regions i,f,o,g); backward-direction input projections are
written in reversed time order (negative-stride matmul rhs) so both chains
use the same slot index.  Biases ride the input projections via a constant
ones row; the Dense head is a final matmul pass.  Host does the cheap
input/output transposes.
"""

import numpy as np
import ml_dtypes
from contextlib import ExitStack

H = 100          # LSTM units per direction
F = 32           # input features
NT = 8           # dense head outputs
T_FULL = 512
B_FULL = 256
N_CORES = 8
B_LOC = B_FULL // N_CORES   # 32
W = 8            # timesteps per PSUM window
# PSUM region order within the 4H axis (host permutes weight columns):
R_I, R_F, R_O, R_G = 0, 1, 2, 3

_BUILD_CACHE = {}
LAST_RESULTS = None  # BassKernelResults of the most recent run (for test.py)


def build_nc(T=T_FULL, B=B_LOC):
    """Build (and bacc-compile) the Bass program for one core."""
    key = (T, B)
    if key in _BUILD_CACHE:
        return _BUILD_CACHE[key]

    import concourse.bacc as bacc
    import concourse.tile as tile
    from concourse import mybir

    fp32 = mybir.dt.float32
    bf16 = mybir.dt.bfloat16
    MAX = mybir.AluOpType.max
    MULT = mybir.AluOpType.mult
    SIG = mybir.ActivationFunctionType.Sigmoid
    RELU = mybir.ActivationFunctionType.Relu

    assert T % W == 0
    TB = T * B
    TB1 = (T + 1) * B
    WB = W * B          # region size (256 cols)

    nc = bacc.Bacc("TRN2", target_bir_lowering=False, debug=False)

    # ---- DRAM I/O ----
    d_x = nc.dram_tensor("x_t", [F + 1, TB], bf16, kind="ExternalInput")
    d_ones = nc.dram_tensor("ones_row", [1, TB1], bf16, kind="ExternalInput")
    dw = {}
    for name, shape in [
        ("wk1f", [F + 1, 4 * H]), ("wr1f", [H, 4 * H]),
        ("wk1b", [F + 1, 4 * H]), ("wr1b", [H, 4 * H]),
        ("wk2f_f", [H + 1, 4 * H]), ("wk2f_b", [H, 4 * H]), ("wr2f", [H, 4 * H]),
        ("wk2b_f", [H + 1, 4 * H]), ("wk2b_b", [H, 4 * H]), ("wr2b", [H, 4 * H]),
        ("wdf", [H + 1, NT]), ("wdb", [H, NT]),
    ]:
        dw[name] = nc.dram_tensor(name, shape, bf16, kind="ExternalInput")
    d_y = nc.dram_tensor("y_t", [NT, TB], fp32, kind="ExternalOutput")

    with tile.TileContext(nc) as tc, ExitStack() as ctx:
        persist = ctx.enter_context(tc.tile_pool(name="persist", bufs=1))
        psum = ctx.enter_context(tc.tile_pool(name="psum", bufs=2, space="PSUM"))
        small = ctx.enter_context(tc.tile_pool(name="small", bufs=4))
        outp = ctx.enter_context(tc.tile_pool(name="outp", bufs=4))

        # ---- persistent SBUF tensors ----
        sb_x = persist.tile([F + 1, TB], bf16, name="sb_x")
        # fwd sequences keep h(phys t) at block t+1 (block 0 = zeros);
        # bwd sequences keep h(phys t) at block t (block T = zeros).
        sb_h1f = persist.tile([H + 1, TB1], bf16, name="sb_h1f")
        sb_h1b = persist.tile([H, TB1], bf16, name="sb_h1b")
        sb_h2f = persist.tile([H + 1, TB1], bf16, name="sb_h2f")
        sb_h2b = persist.tile([H, TB1], bf16, name="sb_h2b")
        # spread the startup DMAs across the four DGE queues — the per-queue
        # HWDGE launch serialization (~625ns each) otherwise delays the
        # first window by ~10us
        qs = [nc.sync, nc.vector, nc.scalar, nc.gpsimd]
        sbw = {}
        for qi, (name, d) in enumerate(dw.items()):
            sbw[name] = persist.tile(list(d.shape), bf16, name="sb_" + name)
            qs[qi % 4].dma_start(out=sbw[name], in_=d.ap())
        nc.sync.dma_start(out=sb_x, in_=d_x.ap())
        # ones rows (bias folding) via DMA (vector memset over 16K cols is slow)
        nc.vector.dma_start(out=sb_h1f[H:H + 1, :], in_=d_ones.ap())
        nc.scalar.dma_start(out=sb_h2f[H:H + 1, :], in_=d_ones.ap())

        # zero-filled "previous [h1,h2]" staging for each layer's first slot.
        # Chain B gets its own copy, rewritten per layer from chain F's
        # slot-0 sigmoid (times zero): the data dependency staggers B's
        # pipeline phase ~700ns behind F so the two chains' engine use
        # interleaves instead of colliding.
        z_stag = persist.tile([H, 2, B], bf16, name="z_stag")
        nc.vector.memset(z_stag, 0.0)
        z_stag_b = persist.tile([H, 2, B], bf16, name="z_stag_b")
        nc.vector.memset(z_stag_b, 0.0)
        cst = {}
        for lname in ("c1f", "c1b", "c2f", "c2b"):
            cst[lname] = persist.tile([H, B], bf16, name=lname)
            nc.vector.memset(cst[lname], 0.0)

        import os
        XCH = int(os.environ.get("BASS_XCH", "4"))  # blocks per xproj matmul
        NCH = W // XCH   # chunks per window region
        HSPLIT = int(os.environ.get("BASS_HSPLIT", "1"))

        def xrhs(src, K, coff, t_lo, reverse, j):
            """Chunk j (XCH blocks) of the window rhs [K, XCH*B]; block
            order reversed for bwd.  Small chunks keep the PE queue
            preemptible so xproj never delays recurrent matmuls long."""
            sl = src[0:K, coff + t_lo * B: coff + (t_lo + W) * B]
            v = sl.rearrange("p (w b) -> p w b", b=B)
            if reverse:
                v = v[:, ::-1, :]
            return v[:, j * XCH:(j + 1) * XCH, :]

        def bilstm(xsrc_f, xsrc_b, wr_f, wr_b, out_f, out_b, c_f, c_b):
            """One bidirectional layer; fwd/bwd as two pipelined chains."""
            nwin = T // W
            chains = [
                dict(cn="f", xsrc=xsrc_f, wr=wr_f, c=c_f, rev=False,
                     stag=z_stag[:, :] if HSPLIT else z_stag[:, 0, :]),
                dict(cn="b", xsrc=xsrc_b, wr=wr_b, c=c_b, rev=True,
                     stag=z_stag_b[:, :] if HSPLIT else z_stag_b[:, 0, :]),
            ]

            def xproj_thunks(w):
                """Input-projection matmul closures for window w.  One PSUM
                tile per chain, regions [i,f,o,g]; per-bank start flags.
                Tiles allocated lazily at first call (emission time)."""
                t0f = w * W
                thb = T - 1 - w * W
                tiles = {}

                def get_tile(cn):
                    if cn not in tiles:
                        tiles[cn] = psum.tile([H, 4 * WB], fp32, bufs=2,
                                              tag="g" + cn, name="g" + cn)
                    return tiles[cn]

                per_chain = []
                for ch in chains:
                    cn = ch["cn"]
                    t_lo = t0f if not ch["rev"] else thb - W + 1
                    started = set()
                    lst = []
                    for r in range(4):
                        for j in range(NCH):
                            off = r * WB + j * XCH * B
                            bank = off // 512
                            for wk, src, K, coff in ch["xsrc"]:
                                first = bank not in started
                                started.add(bank)

                                def thunk(r=r, off=off, j=j, wk=wk, src=src,
                                          K=K, coff=coff, first=first,
                                          t_lo=t_lo, cn=cn, rev=ch["rev"]):
                                    reg = get_tile(cn)
                                    nc.tensor.matmul(
                                        out=reg[:, off:off + XCH * B],
                                        lhsT=wk[:, r * H:(r + 1) * H],
                                        rhs=xrhs(src, K, coff, t_lo, rev, j),
                                        start=first, stop=False,
                                        skip_group_check=True)
                                lst.append(thunk)
                    per_chain.append(lst)
                # interleave F,B so each slot's share touches both chains
                thunks = []
                for pair in zip(*per_chain):
                    thunks.extend(pair)
                return get_tile, thunks

            get_tile, thunks = xproj_thunks(0)
            for th in thunks:
                th()
            for w in range(nwin):
                t0f = w * W                    # fwd slot s -> phys t0f + s
                thb = T - 1 - w * W            # bwd slot s -> phys thb - s
                if w + 1 < nwin:
                    next_get_tile, next_thunks = xproj_thunks(w + 1)
                else:
                    next_get_tile, next_thunks = None, []
                per_slot = (len(next_thunks) + W - 1) // W
                # window-sized [h1,h2] pair staging: feeds next slot's
                # matmuls; summed once per window into wstag then DMA'd to
                # the big sequence buffers
                wpair, wstag = {}, {}
                for ch in chains:
                    if HSPLIT:
                        wpair[ch["cn"]] = small.tile(
                            [H, 2, WB], bf16, tag="wpair" + ch["cn"], bufs=2,
                            name="wpair" + ch["cn"])
                    wstag[ch["cn"]] = small.tile(
                        [H, WB], bf16, tag="wstag" + ch["cn"], bufs=2,
                        name="wstag" + ch["cn"])
                for k in range(W):
                    ksl = slice(k * B, (k + 1) * B)
                    for ch in chains:
                        cn = ch["cn"]
                        reg = get_tile(cn)
                        # recurrent matmuls accumulate Wr^T h via PSUM.
                        # HSPLIT=1: h carried split as [h1,h2] = [so*ct,
                        # so*u] (h = so*c = so*ct + so*u since c >= 0), 8
                        # matmuls, critical path skips the c-add.
                        # HSPLIT=0: plain single h, 4 matmuls.
                        nj = 2 if HSPLIT else 1
                        for r in range(4):
                            for j in range(nj):
                                nc.tensor.matmul(
                                    out=reg[:, r * WB + k * B:
                                            r * WB + (k + 1) * B],
                                    lhsT=ch["wr"][:, r * H:(r + 1) * H],
                                    rhs=ch["stag"][:, j, :] if HSPLIT
                                    else ch["stag"],
                                    start=False, stop=(j == nj - 1),
                                    skip_group_check=True)
                        reg_v = reg.rearrange("p (r n) -> p r n", r=4)
                        # one ACT covers sigmoid of i,f,o (regions 0..2)
                        sig = small.tile([H, 3, B], bf16, tag="sig" + cn,
                                         bufs=2, name="sig" + cn)
                        nc.scalar.activation(sig, reg_v[:, 0:3, ksl], SIG)
                        if w == 0 and k == 0 and cn == "f":
                            # phase-pin: rewrite chain B's zero staging from
                            # F's slot-0 sigmoid (times zero) so B's whole
                            # pipeline starts ~700ns after F
                            nc.vector.scalar_tensor_tensor(
                                out=z_stag_b, in0=sig[:, 0:2, :], scalar=0.0,
                                in1=z_stag, op0=MULT, op1=MULT)
                        # cu = [ct, u]:  ct = sig_f * c ;  u = relu(g)*sig_i
                        cu = small.tile([H, 2, B], bf16, tag="cu" + cn,
                                        bufs=2, name="cu" + cn)
                        nc.vector.tensor_mul(cu[:, 0, :], sig[:, 1, :],
                                             ch["c"])
                        nc.vector.scalar_tensor_tensor(
                            out=cu[:, 1, :], in0=reg_v[:, R_G, ksl],
                            scalar=0.0, in1=sig[:, 0, :], op0=MAX, op1=MULT)
                        if HSPLIT:
                            # paired h-halves: [h1,h2] = sig_o * [ct,u]
                            # written straight into the window pair staging
                            hh = wpair[cn][:, :, ksl]
                            nc.vector.tensor_mul(
                                hh, sig[:, 2:3, :].broadcast_to([H, 2, B]),
                                cu)
                            ch["stag"] = hh
                            # off-critical-path: c = ct + u
                            nc.vector.tensor_add(ch["c"], cu[:, 0, :],
                                                 cu[:, 1, :])
                        else:
                            nc.vector.tensor_add(ch["c"], cu[:, 0, :],
                                                 cu[:, 1, :])
                            nc.vector.tensor_mul(wstag[cn][:, ksl],
                                                 sig[:, 2, :], ch["c"])
                            ch["stag"] = wstag[cn][:, ksl]
                    for th in next_thunks[k * per_slot:(k + 1) * per_slot]:
                        th()
                # one window-batched dense h = h1 + h2 per chain (bf16 2x
                # DVE mode, ~24ns/slot amortized), then flush to the
                # sequence buffers: fwd slots are phys-ascending
                # (contiguous); bwd slots are phys-descending, so reverse
                # the block order on the read side
                if HSPLIT:
                    for ch in chains:
                        cn = ch["cn"]
                        nc.vector.tensor_add(wstag[cn], wpair[cn][:, 0, :],
                                             wpair[cn][:, 1, :])
                nc.sync.dma_start(
                    out=out_f[0:H, (t0f + 1) * B:(t0f + 1 + W) * B],
                    in_=wstag["f"][:, :])
                wstag_b = wstag["b"].rearrange("p (w b) -> p w b", b=B)
                nc.sync.dma_start(
                    out=out_b[0:H, (thb - W + 1) * B:(thb + 1) * B],
                    in_=wstag_b[:, ::-1, :])
                if next_get_tile is not None:
                    get_tile = next_get_tile

        # layer 1: input = x (K = F+1 with bias row)
        bilstm(
            xsrc_f=[(sbw["wk1f"], sb_x, F + 1, 0)],
            xsrc_b=[(sbw["wk1b"], sb_x, F + 1, 0)],
            wr_f=sbw["wr1f"], wr_b=sbw["wr1b"],
            out_f=sb_h1f, out_b=sb_h1b, c_f=cst["c1f"], c_b=cst["c1b"])
        # layer 2: input = [h1f (blocks +1, ones row) ; h1b]
        bilstm(
            xsrc_f=[(sbw["wk2f_f"], sb_h1f, H + 1, B), (sbw["wk2f_b"], sb_h1b, H, 0)],
            xsrc_b=[(sbw["wk2b_f"], sb_h1f, H + 1, B), (sbw["wk2b_b"], sb_h1b, H, 0)],
            wr_f=sbw["wr2f"], wr_b=sbw["wr2b"],
            out_f=sb_h2f, out_b=sb_h2b, c_f=cst["c2f"], c_b=cst["c2b"])

        # dense head: y = relu(Wd^T [h2f;h2b] + bd) over 512-col chunks
        CH = 512
        nch = (TB + CH - 1) // CH
        for ci in range(nch):
            c0 = ci * CH
            n = min(CH, TB - c0)
            ps = psum.tile([NT, CH], fp32, bufs=2, tag="gf", name="ps_y")
            nc.tensor.matmul(out=ps[:, 0:n], lhsT=sbw["wdf"],
                             rhs=sb_h2f[0:H + 1, B + c0: B + c0 + n],
                             start=True, stop=False, skip_group_check=True)
            nc.tensor.matmul(out=ps[:, 0:n], lhsT=sbw["wdb"],
                             rhs=sb_h2b[0:H, c0: c0 + n],
                             start=False, stop=True, skip_group_check=True)
            st = outp.tile([NT, CH], fp32, tag="stage", name="st_y")
            if ci % 2 == 0:
                nc.scalar.activation(st[:, 0:n], ps[:, 0:n], RELU)
            else:
                nc.vector.tensor_single_scalar(st[:, 0:n], ps[:, 0:n], 0.0, MAX)
            nc.sync.dma_start(out=d_y.ap()[:, c0:c0 + n], in_=st[:, 0:n])

    nc.compile()
    _BUILD_CACHE[key] = nc
    return nc


# ---------------------------------------------------------------------------
# host side
# ---------------------------------------------------------------------------

def _bf16(a):
    return np.asarray(a, np.float32).astype(ml_dtypes.bfloat16)


def _perm_gates(w):
    """Reorder 4H gate columns from keras [i,f,g,o] to PSUM [i,f,o,g]."""
    w = np.asarray(w, np.float32)
    i, f, g, o = np.split(w, 4, axis=-1)
    return np.concatenate([i, f, o, g], axis=-1)


def prepare_weight_maps(Wk1f, Wr1f, b1f, Wk1b, Wr1b, b1b,
                        Wk2f, Wr2f, b2f, Wk2b, Wr2b, b2b, Wd, bd):
    def aug(w, b):
        return np.vstack([np.asarray(w, np.float32),
                          np.asarray(b, np.float32)[None, :]])
    P = _perm_gates
    m = {
        "wk1f": P(aug(Wk1f, b1f)), "wr1f": P(Wr1f),
        "wk1b": P(aug(Wk1b, b1b)), "wr1b": P(Wr1b),
        "wk2f_f": P(aug(Wk2f[:H], b2f)), "wk2f_b": P(Wk2f[H:]), "wr2f": P(Wr2f),
        "wk2b_f": P(aug(Wk2b[:H], b2b)), "wk2b_b": P(Wk2b[H:]), "wr2b": P(Wr2b),
        "wdf": aug(Wd[:H], bd), "wdb": Wd[H:],
    }
    return {k: np.ascontiguousarray(_bf16(v)) for k, v in m.items()}


def make_in_maps(x, weights, T, B):
    """x: [Btot, T, F] fp32 -> list of per-core input dicts."""
    n_cores = x.shape[0] // B
    ones = np.ones((1, (T + 1) * B), ml_dtypes.bfloat16)
    in_maps = []
    for c in range(n_cores):
        xc = np.asarray(x[c * B:(c + 1) * B], np.float32)      # [B, T, F]
        xt = xc.transpose(2, 1, 0).reshape(F, T * B)           # [F, T*B] t-major
        x_aug = np.vstack([xt, np.ones((1, T * B), np.float32)])
        in_maps.append({"x_t": np.ascontiguousarray(_bf16(x_aug)),
                        "ones_row": ones, **weights})
    return in_maps


def kernel(x, Wk1f, Wr1f, b1f, Wk1b, Wr1b, b1b,
           Wk2f, Wr2f, b2f, Wk2b, Wr2b, b2b, Wd, bd,
           trace=False):
    global LAST_RESULTS
    from concourse.bass_utils import run_bass_kernel_spmd

    Btot, T, _ = x.shape
    B = Btot // N_CORES
    nc = build_nc(T, B)
    weights = prepare_weight_maps(Wk1f, Wr1f, b1f, Wk1b, Wr1b, b1b,
                                  Wk2f, Wr2f, b2f, Wk2b, Wr2b, b2b, Wd, bd)
    in_maps = make_in_maps(x, weights, T, B)
    res = run_bass_kernel_spmd(nc, in_maps, core_ids=list(range(len(in_maps))),
                               trace=trace)
    LAST_RESULTS = res
    outs = []
    for r in res.results:
        yt = r["y_t"]                                  # [NT, T*B] fp32
        outs.append(yt.reshape(NT, T, B).transpose(2, 1, 0))   # [B, T, NT]
    return np.concatenate(outs, axis=0).astype(np.float32)


# revision 36
# speedup vs baseline: 1.4070x; 1.2446x over previous
"""Trainium2 Bass kernel for a 2-layer BiLSTM regressor (B=256, T=512, F=32,
H=100, relu candidate/output activations, sigmoid gates) + TimeDistributed
Dense(8, relu) head.

Strategy: data-parallel over batch across 8 NeuronCores (32 sequences/core).
Sequences live on-chip transposed as [H, T*B] bf16 tensors (column = t*B+b).

The recurrence is dependency-loop-latency bound (1024 sequential cell steps).
The two directions of a layer run as two INDEPENDENT software-pipelined
chains (not lockstep-fused): each chain's per-step loop is
    PE 4 small matmuls -> ACT one sigmoid over (i,f,o) -> DVE 4 ops -> PE
and the two chains phase-shift so engine busy time hides inside the other
chain's latency segments.  Two exact algebraic simplifications shorten the
loop (c >= 0 by induction since c = sig_f*c + sig_i*relu(g) from c0=0):
    u = relu(g)*sig_i  == one DVE scalar_tensor_tensor (max 0, then mult)
    h = sig_o*relu(c)  == sig_o*c, one plain DVE mult
so no separate relu instructions exist at all.  Gate regions in PSUM are
ordered [i,f,o,g] so one ACT instruction covers all three sigmoids.

Gate pre-activations for an 8-step window land in one 2-bank PSUM tile per
chain ([H, 4*256] regions i,f,o,g); backward-direction input projections are
written in reversed time order (negative-stride matmul rhs) so both chains
use the same slot index.  Biases ride the input projections via a constant
ones row; the Dense head is a final matmul pass.  Host does the cheap
input/output transposes.
"""

import numpy as np
import ml_dtypes
from contextlib import ExitStack

H = 100          # LSTM units per direction
F = 32           # input features
NT = 8           # dense head outputs
T_FULL = 512
B_FULL = 256
N_CORES = 8
B_LOC = B_FULL // N_CORES   # 32
W = 8            # timesteps per PSUM window
# PSUM region order within the 4H axis (host permutes weight columns):
R_I, R_F, R_O, R_G = 0, 1, 2, 3

_BUILD_CACHE = {}
LAST_RESULTS = None  # BassKernelResults of the most recent run (for test.py)


def build_nc(T=T_FULL, B=B_LOC):
    """Build (and bacc-compile) the Bass program for one core."""
    key = (T, B)
    if key in _BUILD_CACHE:
        return _BUILD_CACHE[key]

    import concourse.bacc as bacc
    import concourse.tile as tile
    from concourse import mybir

    fp32 = mybir.dt.float32
    bf16 = mybir.dt.bfloat16
    MAX = mybir.AluOpType.max
    MULT = mybir.AluOpType.mult
    SIG = mybir.ActivationFunctionType.Sigmoid
    RELU = mybir.ActivationFunctionType.Relu

    assert T % W == 0
    TB = T * B
    TB1 = (T + 1) * B
    WB = W * B          # region size (256 cols)

    nc = bacc.Bacc("TRN2", target_bir_lowering=False, debug=False)

    # ---- DRAM I/O ----
    d_x = nc.dram_tensor("x_t", [F + 1, TB], bf16, kind="ExternalInput")
    d_ones = nc.dram_tensor("ones_row", [1, TB1], bf16, kind="ExternalInput")
    dw = {}
    for name, shape in [
        ("wk1f", [F + 1, 4 * H]), ("wr1f", [H, 4 * H]),
        ("wk1b", [F + 1, 4 * H]), ("wr1b", [H, 4 * H]),
        ("wk2f_f", [H + 1, 4 * H]), ("wk2f_b", [H, 4 * H]), ("wr2f", [H, 4 * H]),
        ("wk2b_f", [H + 1, 4 * H]), ("wk2b_b", [H, 4 * H]), ("wr2b", [H, 4 * H]),
        ("wdf", [H + 1, NT]), ("wdb", [H, NT]),
    ]:
        dw[name] = nc.dram_tensor(name, shape, bf16, kind="ExternalInput")
    d_y = nc.dram_tensor("y_t", [NT, TB], fp32, kind="ExternalOutput")

    with tile.TileContext(nc) as tc, ExitStack() as ctx:
        persist = ctx.enter_context(tc.tile_pool(name="persist", bufs=1))
        psum = ctx.enter_context(tc.tile_pool(name="psum", bufs=2, space="PSUM"))
        small = ctx.enter_context(tc.tile_pool(name="small", bufs=4))
        outp = ctx.enter_context(tc.tile_pool(name="outp", bufs=4))

        # ---- persistent SBUF tensors ----
        sb_x = persist.tile([F + 1, TB], bf16, name="sb_x")
        # fwd sequences keep h(phys t) at block t+1 (block 0 = zeros);
        # bwd sequences keep h(phys t) at block t (block T = zeros).
        sb_h1f = persist.tile([H + 1, TB1], bf16, name="sb_h1f")
        sb_h1b = persist.tile([H, TB1], bf16, name="sb_h1b")
        sb_h2f = persist.tile([H + 1, TB1], bf16, name="sb_h2f")
        sb_h2b = persist.tile([H, TB1], bf16, name="sb_h2b")
        # spread the startup DMAs across the four DGE queues — the per-queue
        # HWDGE launch serialization (~625ns each) otherwise delays the
        # first window by ~10us
        qs = [nc.sync, nc.scalar, nc.gpsimd]
        sbw = {}
        for qi, (name, d) in enumerate(dw.items()):
            sbw[name] = persist.tile(list(d.shape), bf16, name="sb_" + name)
            qs[qi % 3].dma_start(out=sbw[name], in_=d.ap())
        nc.sync.dma_start(out=sb_x, in_=d_x.ap())
        # ones rows (bias folding) via DMA (vector memset over 16K cols is slow)
        nc.scalar.dma_start(out=sb_h1f[H:H + 1, :], in_=d_ones.ap())
        nc.gpsimd.dma_start(out=sb_h2f[H:H + 1, :], in_=d_ones.ap())

        # zero-filled "previous [h1,h2]" staging for each layer's first slot.
        # Chain B gets its own copy, rewritten per layer from chain F's
        # slot-0 sigmoid (times zero): the data dependency staggers B's
        # pipeline phase ~700ns behind F so the two chains' engine use
        # interleaves instead of colliding.
        z_stag = persist.tile([H, 2, B], bf16, name="z_stag")
        nc.vector.memset(z_stag, 0.0)
        z_stag_b = persist.tile([H, 2, B], bf16, name="z_stag_b")
        nc.vector.memset(z_stag_b, 0.0)
        cst = {}
        for lname in ("c1f", "c1b", "c2f", "c2b"):
            cst[lname] = persist.tile([H, B], bf16, name=lname)
            nc.vector.memset(cst[lname], 0.0)

        import os
        XCH = int(os.environ.get("BASS_XCH", "4"))  # blocks per xproj matmul
        NCH = W // XCH   # chunks per window region
        HSPLIT = int(os.environ.get("BASS_HSPLIT", "1"))

        def xrhs(src, K, coff, t_lo, reverse, j):
            """Chunk j (XCH blocks) of the window rhs [K, XCH*B]; block
            order reversed for bwd.  Small chunks keep the PE queue
            preemptible so xproj never delays recurrent matmuls long."""
            sl = src[0:K, coff + t_lo * B: coff + (t_lo + W) * B]
            v = sl.rearrange("p (w b) -> p w b", b=B)
            if reverse:
                v = v[:, ::-1, :]
            return v[:, j * XCH:(j + 1) * XCH, :]

        def bilstm(xsrc_f, xsrc_b, wr_f, wr_b, out_f, out_b, c_f, c_b):
            """One bidirectional layer; fwd/bwd as two pipelined chains."""
            nwin = T // W
            chains = [
                dict(cn="f", xsrc=xsrc_f, wr=wr_f, c=c_f, rev=False,
                     stag=z_stag[:, :] if HSPLIT else z_stag[:, 0, :]),
                dict(cn="b", xsrc=xsrc_b, wr=wr_b, c=c_b, rev=True,
                     stag=z_stag_b[:, :] if HSPLIT else z_stag_b[:, 0, :]),
            ]

            def xproj_thunks(w):
                """Input-projection matmul closures for window w.  One PSUM
                tile per chain, regions [i,f,o,g]; per-bank start flags.
                Tiles allocated lazily at first call (emission time)."""
                t0f = w * W
                thb = T - 1 - w * W
                tiles = {}

                def get_tile(cn):
                    if cn not in tiles:
                        tiles[cn] = psum.tile([H, 4 * WB], fp32, bufs=2,
                                              tag="g" + cn, name="g" + cn)
                    return tiles[cn]

                per_chain = []
                for ch in chains:
                    cn = ch["cn"]
                    t_lo = t0f if not ch["rev"] else thb - W + 1
                    started = set()
                    lst = []
                    for r in range(4):
                        for j in range(NCH):
                            off = r * WB + j * XCH * B
                            bank = off // 512
                            for wk, src, K, coff in ch["xsrc"]:
                                first = bank not in started
                                started.add(bank)

                                def thunk(r=r, off=off, j=j, wk=wk, src=src,
                                          K=K, coff=coff, first=first,
                                          t_lo=t_lo, cn=cn, rev=ch["rev"]):
                                    reg = get_tile(cn)
                                    nc.tensor.matmul(
                                        out=reg[:, off:off + XCH * B],
                                        lhsT=wk[:, r * H:(r + 1) * H],
                                        rhs=xrhs(src, K, coff, t_lo, rev, j),
                                        start=first, stop=False,
                                        skip_group_check=True)
                                lst.append(thunk)
                    per_chain.append(lst)
                # interleave F,B so each slot's share touches both chains
                thunks = []
                for pair in zip(*per_chain):
                    thunks.extend(pair)
                return get_tile, thunks

            get_tile, thunks = xproj_thunks(0)
            for th in thunks:
                th()
            for w in range(nwin):
                t0f = w * W                    # fwd slot s -> phys t0f + s
                thb = T - 1 - w * W            # bwd slot s -> phys thb - s
                if w + 1 < nwin:
                    next_get_tile, next_thunks = xproj_thunks(w + 1)
                else:
                    next_get_tile, next_thunks = None, []
                per_slot = (len(next_thunks) + W - 1) // W
                # window-sized [h1,h2] pair staging: feeds next slot's
                # matmuls; summed once per window into wstag then DMA'd to
                # the big sequence buffers
                wpair, wstag = {}, {}
                for ch in chains:
                    if HSPLIT:
                        wpair[ch["cn"]] = small.tile(
                            [H, 2, WB], bf16, tag="wpair" + ch["cn"], bufs=2,
                            name="wpair" + ch["cn"])
                    wstag[ch["cn"]] = small.tile(
                        [H, WB], bf16, tag="wstag" + ch["cn"], bufs=2,
                        name="wstag" + ch["cn"])
                for k in range(W):
                    ksl = slice(k * B, (k + 1) * B)
                    for ch in chains:
                        cn = ch["cn"]
                        reg = get_tile(cn)
                        # recurrent matmuls accumulate Wr^T h via PSUM.
                        # HSPLIT=1: h carried split as [h1,h2] = [so*ct,
                        # so*u] (h = so*c = so*ct + so*u since c >= 0), 8
                        # matmuls, critical path skips the c-add.
                        # HSPLIT=0: plain single h, 4 matmuls.
                        nj = 2 if HSPLIT else 1
                        for r in range(4):
                            for j in range(nj):
                                nc.tensor.matmul(
                                    out=reg[:, r * WB + k * B:
                                            r * WB + (k + 1) * B],
                                    lhsT=ch["wr"][:, r * H:(r + 1) * H],
                                    rhs=ch["stag"][:, j, :] if HSPLIT
                                    else ch["stag"],
                                    start=False, stop=(j == nj - 1),
                                    skip_group_check=True)
                        reg_v = reg.rearrange("p (r n) -> p r n", r=4)
                        # one ACT covers sigmoid of i,f,o (regions 0..2)
                        sig = small.tile([H, 3, B], bf16, tag="sig" + cn,
                                         bufs=2, name="sig" + cn)
                        nc.scalar.activation(sig, reg_v[:, 0:3, ksl], SIG)
                        if w == 0 and k == 0 and cn == "f":
                            # phase-pin: rewrite chain B's zero staging from
                            # F's slot-0 sigmoid (times zero) so B's whole
                            # pipeline starts ~700ns after F
                            nc.vector.scalar_tensor_tensor(
                                out=z_stag_b, in0=sig[:, 0:2, :], scalar=0.0,
                                in1=z_stag, op0=MULT, op1=MULT)
                        # cu = [ct, u]:  ct = sig_f * c ;  u = relu(g)*sig_i
                        cu = small.tile([H, 2, B], bf16, tag="cu" + cn,
                                        bufs=2, name="cu" + cn)
                        nc.vector.tensor_mul(cu[:, 0, :], sig[:, 1, :],
                                             ch["c"])
                        nc.vector.scalar_tensor_tensor(
                            out=cu[:, 1, :], in0=reg_v[:, R_G, ksl],
                            scalar=0.0, in1=sig[:, 0, :], op0=MAX, op1=MULT)
                        if HSPLIT:
                            # paired h-halves: [h1,h2] = sig_o * [ct,u]
                            # written straight into the window pair staging
                            hh = wpair[cn][:, :, ksl]
                            nc.vector.tensor_mul(
                                hh, sig[:, 2:3, :].broadcast_to([H, 2, B]),
                                cu)
                            ch["stag"] = hh
                            # off-critical-path: c = ct + u
                            nc.vector.tensor_add(ch["c"], cu[:, 0, :],
                                                 cu[:, 1, :])
                        else:
                            nc.vector.tensor_add(ch["c"], cu[:, 0, :],
                                                 cu[:, 1, :])
                            nc.vector.tensor_mul(wstag[cn][:, ksl],
                                                 sig[:, 2, :], ch["c"])
                            ch["stag"] = wstag[cn][:, ksl]
                    for th in next_thunks[k * per_slot:(k + 1) * per_slot]:
                        th()
                # one window-batched dense h = h1 + h2 per chain (bf16 2x
                # DVE mode, ~24ns/slot amortized), then flush to the
                # sequence buffers: fwd slots are phys-ascending
                # (contiguous); bwd slots are phys-descending, so reverse
                # the block order on the read side
                if HSPLIT:
                    for ch in chains:
                        cn = ch["cn"]
                        nc.vector.tensor_add(wstag[cn], wpair[cn][:, 0, :],
                                             wpair[cn][:, 1, :])
                nc.sync.dma_start(
                    out=out_f[0:H, (t0f + 1) * B:(t0f + 1 + W) * B],
                    in_=wstag["f"][:, :])
                wstag_b = wstag["b"].rearrange("p (w b) -> p w b", b=B)
                nc.sync.dma_start(
                    out=out_b[0:H, (thb - W + 1) * B:(thb + 1) * B],
                    in_=wstag_b[:, ::-1, :])
                if next_get_tile is not None:
                    get_tile = next_get_tile

        def bilstm_fused(xsrc_f, xsrc_b, wr_f, wr_b, out_f, out_b, c2):
            """Baseline-style lockstep: both directions share every non-PE
            instruction ([H,2,B] tiles).  Fewer instructions per slot than
            the split-chain variant; longer dependency loop.  PSUM regions:
            [i_f,i_b,f_f,f_b,o_f,o_b,g_f,g_b] so one ACT covers all six
            sigmoids and the g-pair is one strided stt view."""
            nwin = T // W
            srcs = {"f": xsrc_f, "b": xsrc_b}
            wrs = {"f": wr_f, "b": wr_b}
            stag = z_stag[:, :, :]      # [H, 2, B] zeros

            def xproj_thunks(w):
                t0f = w * W
                thb = T - 1 - w * W
                tiles = {}

                def get_tile():
                    if "t" not in tiles:
                        tiles["t"] = psum.tile([H, 8 * WB], fp32, bufs=2,
                                               tag="gfb", name="gfb")
                    return tiles["t"]

                thunks = []
                started = set()
                for r in range(4):
                    for di, dn in ((0, "f"), (1, "b")):
                        t_lo = t0f if dn == "f" else thb - W + 1
                        for j in range(NCH):
                            off = (2 * r + di) * WB + j * XCH * B
                            bank = off // 512
                            for wk, src, K, coff in srcs[dn]:
                                first = bank not in started
                                started.add(bank)

                                def thunk(r=r, off=off, j=j, wk=wk, src=src,
                                          K=K, coff=coff, first=first,
                                          t_lo=t_lo, rev=(dn == "b")):
                                    reg = get_tile()
                                    nc.tensor.matmul(
                                        out=reg[:, off:off + XCH * B],
                                        lhsT=wk[:, r * H:(r + 1) * H],
                                        rhs=xrhs(src, K, coff, t_lo, rev, j),
                                        start=first, stop=False,
                                        skip_group_check=True)
                                thunks.append(thunk)
                return get_tile, thunks

            get_tile, thunks = xproj_thunks(0)
            for th in thunks:
                th()
            for w in range(nwin):
                t0f = w * W
                thb = T - 1 - w * W
                if w + 1 < nwin:
                    next_get_tile, next_thunks = xproj_thunks(w + 1)
                else:
                    next_get_tile, next_thunks = None, []
                per_slot = (len(next_thunks) + W - 1) // W
                wstag2 = small.tile([H, 2, WB], bf16, tag="wstag2", bufs=2,
                                    name="wstag2")
                for k in range(W):
                    ksl = slice(k * B, (k + 1) * B)
                    reg = get_tile()
                    for r in range(4):
                        for di, dn in ((0, "f"), (1, "b")):
                            off = (2 * r + di) * WB
                            nc.tensor.matmul(
                                out=reg[:, off + k * B: off + (k + 1) * B],
                                lhsT=wrs[dn][:, r * H:(r + 1) * H],
                                rhs=stag[:, di, :],
                                start=False, stop=True,
                                skip_group_check=True)
                    reg_v = reg.rearrange("p (r n) -> p r n", r=8)
                    sig = small.tile([H, 6, B], bf16, tag="sig2", bufs=2,
                                     name="sig2")
                    nc.scalar.activation(sig, reg_v[:, 0:6, ksl], SIG)
                    cu = small.tile([H, 2, 2, B], bf16, tag="cu2", bufs=2,
                                    name="cu2")
                    # u pair = relu(g pair) * sig_i pair
                    nc.vector.scalar_tensor_tensor(
                        out=cu[:, 1, :, :], in0=reg_v[:, 6:8, ksl],
                        scalar=0.0, in1=sig[:, 0:2, :], op0=MAX, op1=MULT)
                    # ct pair = sig_f pair * c pair ; c = ct + u
                    nc.vector.tensor_mul(cu[:, 0, :, :], sig[:, 2:4, :], c2)
                    nc.vector.tensor_add(c2, cu[:, 0, :, :], cu[:, 1, :, :])
                    # h pair = sig_o pair * c pair -> window staging
                    nc.vector.tensor_mul(wstag2[:, :, ksl], sig[:, 4:6, :],
                                         c2)
                    stag = wstag2[:, :, ksl]
                    for th in next_thunks[k * per_slot:(k + 1) * per_slot]:
                        th()
                nc.sync.dma_start(
                    out=out_f[0:H, (t0f + 1) * B:(t0f + 1 + W) * B],
                    in_=wstag2[:, 0, :])
                wb_v = wstag2.rearrange("p d (w b) -> p d w b", b=B)
                nc.sync.dma_start(
                    out=out_b[0:H, (thb - W + 1) * B:(thb + 1) * B],
                    in_=wb_v[:, 1, ::-1, :])
                if next_get_tile is not None:
                    get_tile = next_get_tile

        FUSED = int(os.environ.get("BASS_FUSED", "0"))
        if FUSED:
            c2l = {}
            for lname in ("cl1", "cl2"):
                c2l[lname] = persist.tile([H, 2, B], bf16, name=lname)
                nc.vector.memset(c2l[lname], 0.0)
            bilstm_fused(
                xsrc_f=[(sbw["wk1f"], sb_x, F + 1, 0)],
                xsrc_b=[(sbw["wk1b"], sb_x, F + 1, 0)],
                wr_f=sbw["wr1f"], wr_b=sbw["wr1b"],
                out_f=sb_h1f, out_b=sb_h1b, c2=c2l["cl1"])
            bilstm_fused(
                xsrc_f=[(sbw["wk2f_f"], sb_h1f, H + 1, B),
                        (sbw["wk2f_b"], sb_h1b, H, 0)],
                xsrc_b=[(sbw["wk2b_f"], sb_h1f, H + 1, B),
                        (sbw["wk2b_b"], sb_h1b, H, 0)],
                wr_f=sbw["wr2f"], wr_b=sbw["wr2b"],
                out_f=sb_h2f, out_b=sb_h2b, c2=c2l["cl2"])
        else:
            # layer 1: input = x (K = F+1 with bias row)
            bilstm(
                xsrc_f=[(sbw["wk1f"], sb_x, F + 1, 0)],
                xsrc_b=[(sbw["wk1b"], sb_x, F + 1, 0)],
                wr_f=sbw["wr1f"], wr_b=sbw["wr1b"],
                out_f=sb_h1f, out_b=sb_h1b, c_f=cst["c1f"], c_b=cst["c1b"])
            # layer 2: input = [h1f (blocks +1, ones row) ; h1b]
            bilstm(
                xsrc_f=[(sbw["wk2f_f"], sb_h1f, H + 1, B),
                        (sbw["wk2f_b"], sb_h1b, H, 0)],
                xsrc_b=[(sbw["wk2b_f"], sb_h1f, H + 1, B),
                        (sbw["wk2b_b"], sb_h1b, H, 0)],
                wr_f=sbw["wr2f"], wr_b=sbw["wr2b"],
                out_f=sb_h2f, out_b=sb_h2b, c_f=cst["c2f"], c_b=cst["c2b"])

        # dense head: y = relu(Wd^T [h2f;h2b] + bd) over 512-col chunks
        CH = 512
        nch = (TB + CH - 1) // CH
        for ci in range(nch):
            c0 = ci * CH
            n = min(CH, TB - c0)
            ps = psum.tile([NT, CH], fp32, bufs=2,
                           tag="gfb" if FUSED else "gf", name="ps_y")
            nc.tensor.matmul(out=ps[:, 0:n], lhsT=sbw["wdf"],
                             rhs=sb_h2f[0:H + 1, B + c0: B + c0 + n],
                             start=True, stop=False, skip_group_check=True)
            nc.tensor.matmul(out=ps[:, 0:n], lhsT=sbw["wdb"],
                             rhs=sb_h2b[0:H, c0: c0 + n],
                             start=False, stop=True, skip_group_check=True)
            st = outp.tile([NT, CH], fp32, tag="stage", name="st_y")
            if ci % 2 == 0:
                nc.scalar.activation(st[:, 0:n], ps[:, 0:n], RELU)
            else:
                nc.vector.tensor_single_scalar(st[:, 0:n], ps[:, 0:n], 0.0, MAX)
            nc.sync.dma_start(out=d_y.ap()[:, c0:c0 + n], in_=st[:, 0:n])

    nc.compile()
    _BUILD_CACHE[key] = nc
    return nc


# ---------------------------------------------------------------------------
# host side
# ---------------------------------------------------------------------------

def _bf16(a):
    return np.asarray(a, np.float32).astype(ml_dtypes.bfloat16)


def _perm_gates(w):
    """Reorder 4H gate columns from keras [i,f,g,o] to PSUM [i,f,o,g]."""
    w = np.asarray(w, np.float32)
    i, f, g, o = np.split(w, 4, axis=-1)
    return np.concatenate([i, f, o, g], axis=-1)


def prepare_weight_maps(Wk1f, Wr1f, b1f, Wk1b, Wr1b, b1b,
                        Wk2f, Wr2f, b2f, Wk2b, Wr2b, b2b, Wd, bd):
    def aug(w, b):
        return np.vstack([np.asarray(w, np.float32),
                          np.asarray(b, np.float32)[None, :]])
    P = _perm_gates
    m = {
        "wk1f": P(aug(Wk1f, b1f)), "wr1f": P(Wr1f),
        "wk1b": P(aug(Wk1b, b1b)), "wr1b": P(Wr1b),
        "wk2f_f": P(aug(Wk2f[:H], b2f)), "wk2f_b": P(Wk2f[H:]), "wr2f": P(Wr2f),
        "wk2b_f": P(aug(Wk2b[:H], b2b)), "wk2b_b": P(Wk2b[H:]), "wr2b": P(Wr2b),
        "wdf": aug(Wd[:H], bd), "wdb": Wd[H:],
    }
    return {k: np.ascontiguousarray(_bf16(v)) for k, v in m.items()}


def make_in_maps(x, weights, T, B):
    """x: [Btot, T, F] fp32 -> list of per-core input dicts."""
    n_cores = x.shape[0] // B
    ones = np.ones((1, (T + 1) * B), ml_dtypes.bfloat16)
    in_maps = []
    for c in range(n_cores):
        xc = np.asarray(x[c * B:(c + 1) * B], np.float32)      # [B, T, F]
        xt = xc.transpose(2, 1, 0).reshape(F, T * B)           # [F, T*B] t-major
        x_aug = np.vstack([xt, np.ones((1, T * B), np.float32)])
        in_maps.append({"x_t": np.ascontiguousarray(_bf16(x_aug)),
                        "ones_row": ones, **weights})
    return in_maps


def kernel(x, Wk1f, Wr1f, b1f, Wk1b, Wr1b, b1b,
           Wk2f, Wr2f, b2f, Wk2b, Wr2b, b2b, Wd, bd,
           trace=False):
    global LAST_RESULTS
    from concourse.bass_utils import run_bass_kernel_spmd

    Btot, T, _ = x.shape
    B = Btot // N_CORES
    nc = build_nc(T, B)
    weights = prepare_weight_maps(Wk1f, Wr1f, b1f, Wk1b, Wr1b, b1b,
                                  Wk2f, Wr2f, b2f, Wk2b, Wr2b, b2b, Wd, bd)
    in_maps = make_in_maps(x, weights, T, B)
    res = run_bass_kernel_spmd(nc, in_maps, core_ids=list(range(len(in_maps))),
                               trace=trace)
    LAST_RESULTS = res
    outs = []
    for r in res.results:
        yt = r["y_t"]                                  # [NT, T*B] fp32
        outs.append(yt.reshape(NT, T, B).transpose(2, 1, 0))   # [B, T, NT]
    return np.concatenate(outs, axis=0).astype(np.float32)


# revision 38
# speedup vs baseline: 1.6074x; 1.1424x over previous
"""Trainium2 Bass kernel for a 2-layer BiLSTM regressor (B=256, T=512, F=32,
H=100, relu candidate/output activations, sigmoid gates) + TimeDistributed
Dense(8, relu) head.

Strategy: data-parallel over batch across 8 NeuronCores (32 sequences/core).
Sequences live on-chip transposed as [H, T*B] bf16 tensors (column = t*B+b).

The recurrence is dependency-loop-latency bound (1024 sequential cell steps).
The two directions of a layer run as two INDEPENDENT software-pipelined
chains (not lockstep-fused): each chain's per-step loop is
    PE 4 small matmuls -> ACT one sigmoid over (i,f,o) -> DVE 4 ops -> PE
and the two chains phase-shift so engine busy time hides inside the other
chain's latency segments.  Two exact algebraic simplifications shorten the
loop (c >= 0 by induction since c = sig_f*c + sig_i*relu(g) from c0=0):
    u = relu(g)*sig_i  == one DVE scalar_tensor_tensor (max 0, then mult)
    h = sig_o*relu(c)  == sig_o*c, one plain DVE mult
so no separate relu instructions exist at all.  Gate regions in PSUM are
ordered [i,f,o,g] so one ACT instruction covers all three sigmoids.

Gate pre-activations for an 8-step window land in one 2-bank PSUM tile per
chain ([H, 4*256] regions i,f,o,g); backward-direction input projections are
written in reversed time order (negative-stride matmul rhs) so both chains
use the same slot index.  Biases ride the input projections via a constant
ones row; the Dense head is a final matmul pass.  Host does the cheap
input/output transposes.
"""

import numpy as np
import ml_dtypes
from contextlib import ExitStack

H = 100          # LSTM units per direction
F = 32           # input features
NT = 8           # dense head outputs
T_FULL = 512
B_FULL = 256
N_CORES = 8
B_LOC = B_FULL // N_CORES   # 32
W = 8            # timesteps per PSUM window
# PSUM region order within the 4H axis (host permutes weight columns):
R_I, R_F, R_O, R_G = 0, 1, 2, 3

_BUILD_CACHE = {}
LAST_RESULTS = None  # BassKernelResults of the most recent run (for test.py)


def build_nc(T=T_FULL, B=B_LOC):
    """Build (and bacc-compile) the Bass program for one core."""
    key = (T, B)
    if key in _BUILD_CACHE:
        return _BUILD_CACHE[key]

    import concourse.bacc as bacc
    import concourse.tile as tile
    from concourse import mybir

    fp32 = mybir.dt.float32
    bf16 = mybir.dt.bfloat16
    MAX = mybir.AluOpType.max
    MULT = mybir.AluOpType.mult
    SIG = mybir.ActivationFunctionType.Sigmoid
    RELU = mybir.ActivationFunctionType.Relu

    assert T % W == 0
    TB = T * B
    TB1 = (T + 1) * B
    WB = W * B          # region size (256 cols)

    nc = bacc.Bacc("TRN2", target_bir_lowering=False, debug=False)

    # ---- DRAM I/O ----
    d_x = nc.dram_tensor("x_t", [F + 1, TB], bf16, kind="ExternalInput")
    d_ones = nc.dram_tensor("ones_row", [1, TB1], bf16, kind="ExternalInput")
    dw = {}
    for name, shape in [
        ("wk1f", [F + 1, 4 * H]), ("wr1f", [H, 4 * H]),
        ("wk1b", [F + 1, 4 * H]), ("wr1b", [H, 4 * H]),
        ("wk2f_f", [H + 1, 4 * H]), ("wk2f_b", [H, 4 * H]), ("wr2f", [H, 4 * H]),
        ("wk2b_f", [H + 1, 4 * H]), ("wk2b_b", [H, 4 * H]), ("wr2b", [H, 4 * H]),
        ("wdf", [H + 1, NT]), ("wdb", [H, NT]),
    ]:
        dw[name] = nc.dram_tensor(name, shape, bf16, kind="ExternalInput")
    d_y = nc.dram_tensor("y_t", [NT, TB], fp32, kind="ExternalOutput")

    with tile.TileContext(nc) as tc, ExitStack() as ctx:
        persist = ctx.enter_context(tc.tile_pool(name="persist", bufs=1))
        psum = ctx.enter_context(tc.tile_pool(name="psum", bufs=2, space="PSUM"))
        small = ctx.enter_context(tc.tile_pool(name="small", bufs=4))
        outp = ctx.enter_context(tc.tile_pool(name="outp", bufs=4))

        # ---- persistent SBUF tensors ----
        sb_x = persist.tile([F + 1, TB], bf16, name="sb_x")
        # fwd sequences keep h(phys t) at block t+1 (block 0 = zeros);
        # bwd sequences keep h(phys t) at block t (block T = zeros).
        sb_h1f = persist.tile([H + 1, TB1], bf16, name="sb_h1f")
        sb_h1b = persist.tile([H, TB1], bf16, name="sb_h1b")
        sb_h2f = persist.tile([H + 1, TB1], bf16, name="sb_h2f")
        sb_h2b = persist.tile([H, TB1], bf16, name="sb_h2b")
        # spread the startup DMAs across the four DGE queues — the per-queue
        # HWDGE launch serialization (~625ns each) otherwise delays the
        # first window by ~10us
        qs = [nc.sync, nc.scalar, nc.gpsimd]
        sbw = {}
        for qi, (name, d) in enumerate(dw.items()):
            sbw[name] = persist.tile(list(d.shape), bf16, name="sb_" + name)
            qs[qi % 3].dma_start(out=sbw[name], in_=d.ap())
        nc.sync.dma_start(out=sb_x, in_=d_x.ap())
        # ones rows (bias folding) via DMA (vector memset over 16K cols is slow)
        nc.scalar.dma_start(out=sb_h1f[H:H + 1, :], in_=d_ones.ap())
        nc.gpsimd.dma_start(out=sb_h2f[H:H + 1, :], in_=d_ones.ap())

        # zero-filled "previous [h1,h2]" staging for each layer's first slot.
        # Chain B gets its own copy, rewritten per layer from chain F's
        # slot-0 sigmoid (times zero): the data dependency staggers B's
        # pipeline phase ~700ns behind F so the two chains' engine use
        # interleaves instead of colliding.
        z_stag = persist.tile([H, 2, B], bf16, name="z_stag")
        nc.vector.memset(z_stag, 0.0)
        z_stag_b = persist.tile([H, 2, B], bf16, name="z_stag_b")
        nc.vector.memset(z_stag_b, 0.0)
        cst = {}
        for lname in ("c1f", "c1b", "c2f", "c2b"):
            cst[lname] = persist.tile([H, B], bf16, name=lname)
            nc.vector.memset(cst[lname], 0.0)

        import os
        XCH = int(os.environ.get("BASS_XCH", "8"))  # blocks per xproj matmul (8 = unchunked)
        NCH = W // XCH   # chunks per window region
        HSPLIT = int(os.environ.get("BASS_HSPLIT", "0"))

        def xrhs(src, K, coff, t_lo, reverse, j):
            """Chunk j (XCH blocks) of the window rhs [K, XCH*B]; block
            order reversed for bwd.  Small chunks keep the PE queue
            preemptible so xproj never delays recurrent matmuls long."""
            sl = src[0:K, coff + t_lo * B: coff + (t_lo + W) * B]
            v = sl.rearrange("p (w b) -> p w b", b=B)
            if reverse:
                v = v[:, ::-1, :]
            return v[:, j * XCH:(j + 1) * XCH, :]

        def bilstm(xsrc_f, xsrc_b, wr_f, wr_b, out_f, out_b, c_f, c_b):
            """One bidirectional layer; fwd/bwd as two pipelined chains."""
            nwin = T // W
            chains = [
                dict(cn="f", xsrc=xsrc_f, wr=wr_f, c=c_f, rev=False,
                     stag=z_stag[:, :] if HSPLIT else z_stag[:, 0, :]),
                dict(cn="b", xsrc=xsrc_b, wr=wr_b, c=c_b, rev=True,
                     stag=z_stag_b[:, :] if HSPLIT else z_stag_b[:, 0, :]),
            ]

            def xproj_thunks(w):
                """Input-projection matmul closures for window w.  One PSUM
                tile per chain, regions [i,f,o,g]; per-bank start flags.
                Tiles allocated lazily at first call (emission time)."""
                t0f = w * W
                thb = T - 1 - w * W
                tiles = {}

                def get_tile(cn):
                    if cn not in tiles:
                        tiles[cn] = psum.tile([H, 4 * WB], fp32, bufs=2,
                                              tag="g" + cn, name="g" + cn)
                    return tiles[cn]

                per_chain = []
                for ch in chains:
                    cn = ch["cn"]
                    t_lo = t0f if not ch["rev"] else thb - W + 1
                    started = set()
                    lst = []
                    for r in range(4):
                        for j in range(NCH):
                            off = r * WB + j * XCH * B
                            bank = off // 512
                            for wk, src, K, coff in ch["xsrc"]:
                                first = bank not in started
                                started.add(bank)

                                def thunk(r=r, off=off, j=j, wk=wk, src=src,
                                          K=K, coff=coff, first=first,
                                          t_lo=t_lo, cn=cn, rev=ch["rev"]):
                                    reg = get_tile(cn)
                                    nc.tensor.matmul(
                                        out=reg[:, off:off + XCH * B],
                                        lhsT=wk[:, r * H:(r + 1) * H],
                                        rhs=xrhs(src, K, coff, t_lo, rev, j),
                                        start=first, stop=False,
                                        skip_group_check=True)
                                lst.append(thunk)
                    per_chain.append(lst)
                # interleave F,B so each slot's share touches both chains
                thunks = []
                for pair in zip(*per_chain):
                    thunks.extend(pair)
                return get_tile, thunks

            get_tile, thunks = xproj_thunks(0)
            for th in thunks:
                th()
            for w in range(nwin):
                t0f = w * W                    # fwd slot s -> phys t0f + s
                thb = T - 1 - w * W            # bwd slot s -> phys thb - s
                if w + 1 < nwin:
                    next_get_tile, next_thunks = xproj_thunks(w + 1)
                else:
                    next_get_tile, next_thunks = None, []
                per_slot = (len(next_thunks) + W - 1) // W
                # window-sized [h1,h2] pair staging: feeds next slot's
                # matmuls; summed once per window into wstag then DMA'd to
                # the big sequence buffers
                wpair, wstag = {}, {}
                for ch in chains:
                    if HSPLIT:
                        wpair[ch["cn"]] = small.tile(
                            [H, 2, WB], bf16, tag="wpair" + ch["cn"], bufs=2,
                            name="wpair" + ch["cn"])
                    wstag[ch["cn"]] = small.tile(
                        [H, WB], bf16, tag="wstag" + ch["cn"], bufs=2,
                        name="wstag" + ch["cn"])
                for k in range(W):
                    ksl = slice(k * B, (k + 1) * B)
                    for ch in chains:
                        cn = ch["cn"]
                        reg = get_tile(cn)
                        # recurrent matmuls accumulate Wr^T h via PSUM.
                        # HSPLIT=1: h carried split as [h1,h2] = [so*ct,
                        # so*u] (h = so*c = so*ct + so*u since c >= 0), 8
                        # matmuls, critical path skips the c-add.
                        # HSPLIT=0: plain single h, 4 matmuls.
                        nj = 2 if HSPLIT else 1
                        for r in range(4):
                            for j in range(nj):
                                nc.tensor.matmul(
                                    out=reg[:, r * WB + k * B:
                                            r * WB + (k + 1) * B],
                                    lhsT=ch["wr"][:, r * H:(r + 1) * H],
                                    rhs=ch["stag"][:, j, :] if HSPLIT
                                    else ch["stag"],
                                    start=False, stop=(j == nj - 1),
                                    skip_group_check=True)
                        reg_v = reg.rearrange("p (r n) -> p r n", r=4)
                        # one ACT covers sigmoid of i,f,o (regions 0..2)
                        sig = small.tile([H, 3, B], bf16, tag="sig" + cn,
                                         bufs=2, name="sig" + cn)
                        nc.scalar.activation(sig, reg_v[:, 0:3, ksl], SIG)
                        if w == 0 and k == 0 and cn == "f":
                            # phase-pin: rewrite chain B's zero staging from
                            # F's slot-0 sigmoid (times zero) so B's whole
                            # pipeline starts ~700ns after F
                            nc.vector.scalar_tensor_tensor(
                                out=z_stag_b, in0=sig[:, 0:2, :], scalar=0.0,
                                in1=z_stag, op0=MULT, op1=MULT)
                        # cu = [ct, u]:  ct = sig_f * c ;  u = relu(g)*sig_i
                        cu = small.tile([H, 2, B], bf16, tag="cu" + cn,
                                        bufs=2, name="cu" + cn)
                        nc.vector.tensor_mul(cu[:, 0, :], sig[:, 1, :],
                                             ch["c"])
                        nc.vector.scalar_tensor_tensor(
                            out=cu[:, 1, :], in0=reg_v[:, R_G, ksl],
                            scalar=0.0, in1=sig[:, 0, :], op0=MAX, op1=MULT)
                        if HSPLIT:
                            # paired h-halves: [h1,h2] = sig_o * [ct,u]
                            # written straight into the window pair staging
                            hh = wpair[cn][:, :, ksl]
                            nc.vector.tensor_mul(
                                hh, sig[:, 2:3, :].broadcast_to([H, 2, B]),
                                cu)
                            ch["stag"] = hh
                            # off-critical-path: c = ct + u
                            nc.vector.tensor_add(ch["c"], cu[:, 0, :],
                                                 cu[:, 1, :])
                        else:
                            nc.vector.tensor_add(ch["c"], cu[:, 0, :],
                                                 cu[:, 1, :])
                            nc.vector.tensor_mul(wstag[cn][:, ksl],
                                                 sig[:, 2, :], ch["c"])
                            ch["stag"] = wstag[cn][:, ksl]
                    for th in next_thunks[k * per_slot:(k + 1) * per_slot]:
                        th()
                # one window-batched dense h = h1 + h2 per chain (bf16 2x
                # DVE mode, ~24ns/slot amortized), then flush to the
                # sequence buffers: fwd slots are phys-ascending
                # (contiguous); bwd slots are phys-descending, so reverse
                # the block order on the read side
                if HSPLIT:
                    for ch in chains:
                        cn = ch["cn"]
                        nc.vector.tensor_add(wstag[cn], wpair[cn][:, 0, :],
                                             wpair[cn][:, 1, :])
                nc.sync.dma_start(
                    out=out_f[0:H, (t0f + 1) * B:(t0f + 1 + W) * B],
                    in_=wstag["f"][:, :])
                wstag_b = wstag["b"].rearrange("p (w b) -> p w b", b=B)
                nc.sync.dma_start(
                    out=out_b[0:H, (thb - W + 1) * B:(thb + 1) * B],
                    in_=wstag_b[:, ::-1, :])
                if next_get_tile is not None:
                    get_tile = next_get_tile

        def bilstm_fused(xsrc_f, xsrc_b, wr_f, wr_b, out_f, out_b, c2):
            """Baseline-style lockstep: both directions share every non-PE
            instruction ([H,2,B] tiles).  Fewer instructions per slot than
            the split-chain variant; longer dependency loop.  PSUM regions:
            [i_f,i_b,f_f,f_b,o_f,o_b,g_f,g_b] so one ACT covers all six
            sigmoids and the g-pair is one strided stt view."""
            nwin = T // W
            srcs = {"f": xsrc_f, "b": xsrc_b}
            wrs = {"f": wr_f, "b": wr_b}
            stag = z_stag[:, :, :]      # [H, 2, B] zeros

            def xproj_thunks(w):
                t0f = w * W
                thb = T - 1 - w * W
                tiles = {}

                def get_tile():
                    if "t" not in tiles:
                        tiles["t"] = psum.tile([H, 8 * WB], fp32, bufs=2,
                                               tag="gfb", name="gfb")
                    return tiles["t"]

                thunks = []
                started = set()
                for r in range(4):
                    for di, dn in ((0, "f"), (1, "b")):
                        t_lo = t0f if dn == "f" else thb - W + 1
                        for j in range(NCH):
                            off = (2 * r + di) * WB + j * XCH * B
                            bank = off // 512
                            for wk, src, K, coff in srcs[dn]:
                                first = bank not in started
                                started.add(bank)

                                def thunk(r=r, off=off, j=j, wk=wk, src=src,
                                          K=K, coff=coff, first=first,
                                          t_lo=t_lo, rev=(dn == "b")):
                                    reg = get_tile()
                                    nc.tensor.matmul(
                                        out=reg[:, off:off + XCH * B],
                                        lhsT=wk[:, r * H:(r + 1) * H],
                                        rhs=xrhs(src, K, coff, t_lo, rev, j),
                                        start=first, stop=False,
                                        skip_group_check=True)
                                thunks.append(thunk)
                return get_tile, thunks

            get_tile, thunks = xproj_thunks(0)
            for th in thunks:
                th()
            for w in range(nwin):
                t0f = w * W
                thb = T - 1 - w * W
                if w + 1 < nwin:
                    next_get_tile, next_thunks = xproj_thunks(w + 1)
                else:
                    next_get_tile, next_thunks = None, []
                per_slot = (len(next_thunks) + W - 1) // W
                wstag2 = small.tile([H, 2, WB], bf16, tag="wstag2", bufs=2,
                                    name="wstag2")
                for k in range(W):
                    ksl = slice(k * B, (k + 1) * B)
                    reg = get_tile()
                    for r in range(4):
                        for di, dn in ((0, "f"), (1, "b")):
                            off = (2 * r + di) * WB
                            nc.tensor.matmul(
                                out=reg[:, off + k * B: off + (k + 1) * B],
                                lhsT=wrs[dn][:, r * H:(r + 1) * H],
                                rhs=stag[:, di, :],
                                start=False, stop=True,
                                skip_group_check=True)
                    reg_v = reg.rearrange("p (r n) -> p r n", r=8)
                    sig = small.tile([H, 6, B], bf16, tag="sig2", bufs=2,
                                     name="sig2")
                    nc.scalar.activation(sig, reg_v[:, 0:6, ksl], SIG)
                    cu = small.tile([H, 2, 2, B], bf16, tag="cu2", bufs=2,
                                    name="cu2")
                    # u pair = relu(g pair) * sig_i pair
                    nc.vector.scalar_tensor_tensor(
                        out=cu[:, 1, :, :], in0=reg_v[:, 6:8, ksl],
                        scalar=0.0, in1=sig[:, 0:2, :], op0=MAX, op1=MULT)
                    # ct pair = sig_f pair * c pair ; c = ct + u
                    nc.vector.tensor_mul(cu[:, 0, :, :], sig[:, 2:4, :], c2)
                    nc.vector.tensor_add(c2, cu[:, 0, :, :], cu[:, 1, :, :])
                    # h pair = sig_o pair * c pair -> window staging
                    nc.vector.tensor_mul(wstag2[:, :, ksl], sig[:, 4:6, :],
                                         c2)
                    stag = wstag2[:, :, ksl]
                    for th in next_thunks[k * per_slot:(k + 1) * per_slot]:
                        th()
                nc.sync.dma_start(
                    out=out_f[0:H, (t0f + 1) * B:(t0f + 1 + W) * B],
                    in_=wstag2[:, 0, :])
                wb_v = wstag2.rearrange("p d (w b) -> p d w b", b=B)
                nc.sync.dma_start(
                    out=out_b[0:H, (thb - W + 1) * B:(thb + 1) * B],
                    in_=wb_v[:, 1, ::-1, :])
                if next_get_tile is not None:
                    get_tile = next_get_tile

        FUSED = int(os.environ.get("BASS_FUSED", "0"))
        if FUSED:
            c2l = {}
            for lname in ("cl1", "cl2"):
                c2l[lname] = persist.tile([H, 2, B], bf16, name=lname)
                nc.vector.memset(c2l[lname], 0.0)
            bilstm_fused(
                xsrc_f=[(sbw["wk1f"], sb_x, F + 1, 0)],
                xsrc_b=[(sbw["wk1b"], sb_x, F + 1, 0)],
                wr_f=sbw["wr1f"], wr_b=sbw["wr1b"],
                out_f=sb_h1f, out_b=sb_h1b, c2=c2l["cl1"])
            bilstm_fused(
                xsrc_f=[(sbw["wk2f_f"], sb_h1f, H + 1, B),
                        (sbw["wk2f_b"], sb_h1b, H, 0)],
                xsrc_b=[(sbw["wk2b_f"], sb_h1f, H + 1, B),
                        (sbw["wk2b_b"], sb_h1b, H, 0)],
                wr_f=sbw["wr2f"], wr_b=sbw["wr2b"],
                out_f=sb_h2f, out_b=sb_h2b, c2=c2l["cl2"])
        else:
            # layer 1: input = x (K = F+1 with bias row)
            bilstm(
                xsrc_f=[(sbw["wk1f"], sb_x, F + 1, 0)],
                xsrc_b=[(sbw["wk1b"], sb_x, F + 1, 0)],
                wr_f=sbw["wr1f"], wr_b=sbw["wr1b"],
                out_f=sb_h1f, out_b=sb_h1b, c_f=cst["c1f"], c_b=cst["c1b"])
            # layer 2: input = [h1f (blocks +1, ones row) ; h1b]
            bilstm(
                xsrc_f=[(sbw["wk2f_f"], sb_h1f, H + 1, B),
                        (sbw["wk2f_b"], sb_h1b, H, 0)],
                xsrc_b=[(sbw["wk2b_f"], sb_h1f, H + 1, B),
                        (sbw["wk2b_b"], sb_h1b, H, 0)],
                wr_f=sbw["wr2f"], wr_b=sbw["wr2b"],
                out_f=sb_h2f, out_b=sb_h2b, c_f=cst["c2f"], c_b=cst["c2b"])

        # dense head: y = relu(Wd^T [h2f;h2b] + bd) over 512-col chunks
        CH = 512
        nch = (TB + CH - 1) // CH
        for ci in range(nch):
            c0 = ci * CH
            n = min(CH, TB - c0)
            ps = psum.tile([NT, CH], fp32, bufs=2,
                           tag="gfb" if FUSED else "gf", name="ps_y")
            nc.tensor.matmul(out=ps[:, 0:n], lhsT=sbw["wdf"],
                             rhs=sb_h2f[0:H + 1, B + c0: B + c0 + n],
                             start=True, stop=False, skip_group_check=True)
            nc.tensor.matmul(out=ps[:, 0:n], lhsT=sbw["wdb"],
                             rhs=sb_h2b[0:H, c0: c0 + n],
                             start=False, stop=True, skip_group_check=True)
            st = outp.tile([NT, CH], fp32, tag="stage", name="st_y")
            if ci % 2 == 0:
                nc.scalar.activation(st[:, 0:n], ps[:, 0:n], RELU)
            else:
                nc.vector.tensor_single_scalar(st[:, 0:n], ps[:, 0:n], 0.0, MAX)
            nc.sync.dma_start(out=d_y.ap()[:, c0:c0 + n], in_=st[:, 0:n])

    nc.compile()
    _BUILD_CACHE[key] = nc
    return nc


# ---------------------------------------------------------------------------
# host side
# ---------------------------------------------------------------------------

def _bf16(a):
    return np.asarray(a, np.float32).astype(ml_dtypes.bfloat16)


def _perm_gates(w):
    """Reorder 4H gate columns from keras [i,f,g,o] to PSUM [i,f,o,g]."""
    w = np.asarray(w, np.float32)
    i, f, g, o = np.split(w, 4, axis=-1)
    return np.concatenate([i, f, o, g], axis=-1)


def prepare_weight_maps(Wk1f, Wr1f, b1f, Wk1b, Wr1b, b1b,
                        Wk2f, Wr2f, b2f, Wk2b, Wr2b, b2b, Wd, bd):
    def aug(w, b):
        return np.vstack([np.asarray(w, np.float32),
                          np.asarray(b, np.float32)[None, :]])
    P = _perm_gates
    m = {
        "wk1f": P(aug(Wk1f, b1f)), "wr1f": P(Wr1f),
        "wk1b": P(aug(Wk1b, b1b)), "wr1b": P(Wr1b),
        "wk2f_f": P(aug(Wk2f[:H], b2f)), "wk2f_b": P(Wk2f[H:]), "wr2f": P(Wr2f),
        "wk2b_f": P(aug(Wk2b[:H], b2b)), "wk2b_b": P(Wk2b[H:]), "wr2b": P(Wr2b),
        "wdf": aug(Wd[:H], bd), "wdb": Wd[H:],
    }
    return {k: np.ascontiguousarray(_bf16(v)) for k, v in m.items()}


def make_in_maps(x, weights, T, B):
    """x: [Btot, T, F] fp32 -> list of per-core input dicts."""
    n_cores = x.shape[0] // B
    ones = np.ones((1, (T + 1) * B), ml_dtypes.bfloat16)
    in_maps = []
    for c in range(n_cores):
        xc = np.asarray(x[c * B:(c + 1) * B], np.float32)      # [B, T, F]
        xt = xc.transpose(2, 1, 0).reshape(F, T * B)           # [F, T*B] t-major
        x_aug = np.vstack([xt, np.ones((1, T * B), np.float32)])
        in_maps.append({"x_t": np.ascontiguousarray(_bf16(x_aug)),
                        "ones_row": ones, **weights})
    return in_maps


def kernel(x, Wk1f, Wr1f, b1f, Wk1b, Wr1b, b1b,
           Wk2f, Wr2f, b2f, Wk2b, Wr2b, b2b, Wd, bd,
           trace=False):
    global LAST_RESULTS
    from concourse.bass_utils import run_bass_kernel_spmd

    Btot, T, _ = x.shape
    B = Btot // N_CORES
    nc = build_nc(T, B)
    weights = prepare_weight_maps(Wk1f, Wr1f, b1f, Wk1b, Wr1b, b1b,
                                  Wk2f, Wr2f, b2f, Wk2b, Wr2b, b2b, Wd, bd)
    in_maps = make_in_maps(x, weights, T, B)
    res = run_bass_kernel_spmd(nc, in_maps, core_ids=list(range(len(in_maps))),
                               trace=trace)
    LAST_RESULTS = res
    outs = []
    for r in res.results:
        yt = r["y_t"]                                  # [NT, T*B] fp32
        outs.append(yt.reshape(NT, T, B).transpose(2, 1, 0))   # [B, T, NT]
    return np.concatenate(outs, axis=0).astype(np.float32)
